# revision 1
# baseline (speedup 1.0000x reference)
"""Trainium2 Bass kernel for a 2-layer dense GCN (NodeEncoder).

    out = adj @ relu(adj @ (x@W1) + b1) @ W2 + b2
    N=16384, F_IN=512, HID=1024, OUT=256, adj dense [N, N] fp32.

Sharding: adj row-partitioned across 8 NeuronCores (2048 rows/core).
All device matmuls consume natural-layout (row-major) operands; the
host pre-transposes adj/x per shard so no on-device transposes are
needed.  Per core:

  phase A:  s1_c   = x_c @ W1                  [2048, 1024]  (own rows)
  AG1:      s1     = AllGather(s1_c)           [16384, 1024]
  phase B:  hT_c   = relu(adj_c @ s1 + b1)^T   [1024, 2048]  (transposed
            orientation: lhsT = s1 tiles, rhs = adjT_c tiles -> psum is
            [n, m]; bias b1 is per-partition, fused into the ACT relu)
  phase C:  s2_c   = h_c @ W2                  [2048, 256]   (lhsT = hT_c)
  AG2:      s2     = AllGather(s2_c)           [16384, 256]
  phase D:  out2T_c = (adj_c @ s2)^T + b2      [256, 2048]   (lhsT = s2
            tiles, rhs = adjT_c tiles; b2 per-partition via ACT Copy)

Matmuls run in bf16 with fp32 PSUM accumulation (max rel err vs fp32
reference ~3e-3 of absmax).
"""

import numpy as np
import ml_dtypes

import concourse.bass as bass
import concourse.mybir as mybir
import concourse.tile as tile
from concourse.bass_utils import run_bass_kernel_spmd
from concourse.tile_sem_assignment import N_PROCS
from concourse.vector_clock import ScopedClock, VectorClock
from concourse.tile_rust import add_dep_helper as tile_rust_add_dep

# ---------------------------------------------------------------------------
# Workaround: the walrus build in this container caps the number of sync-wait
# commands on a Drain instruction; Tile's kernel-tail drain aggregates one
# wait per logical processor and exceeds it.  Split the tail drain into a
# chain of single-wait drains on the same (SP) queue — semantically identical.
# ---------------------------------------------------------------------------


def _drain_and_barrier_split(self, tick_clock, wait_clock):
    gc = tick_clock.global_clock
    for p in range(N_PROCS):
        partial = VectorClock([gc[q] if q == p else 0 for q in range(N_PROCS)])
        d = self.nc.sync.drain()
        wait_clock.add_sem_waits(d.ins, ScopedClock({None: partial}))
    self.nc.sync.drain()

    self.nc.all_engine_barrier()
    assert self.sems is not None
    popped = self.nc._tile_sem_poison_stack.pop()
    assert popped is self._sem_poison
    self.nc.clear_and_free_semaphores(list(self.sems.allocated().values()))
    self.nc.all_engine_barrier()


tile.TileContext._drain_and_barrier = _drain_and_barrier_split

# The same walrus cap applies to every instruction kind: at most ONE sync
# wait command per instruction (probed empirically — a 2-wait TensorCopy is
# rejected).  Post-pass: hoist excess sem-waits onto no-ops inserted just
# before the instruction on the same engine queue — per-engine program order
# makes this semantically identical.
_MAX_WAITS = 1


def _split_excess_waits(nc):
    ctr = 0
    for f in nc.m.functions:
        for bb in f.blocks:
            out = []
            changed = False
            for inst in bb.instructions:
                si = inst.sync_info
                waits = list(si.on_wait) if si is not None and si.on_wait else []
                if len(waits) > _MAX_WAITS:
                    changed = True
                    keep, excess = waits[: _MAX_WAITS], waits[_MAX_WAITS :]
                    for i in range(0, len(excess), _MAX_WAITS):
                        ctr += 1
                        nop = mybir.InstNoOp(name=f"I-waitnop-{ctr}")
                        nop.engine = inst.engine
                        nop.sync_info = mybir.SyncInfo(
                            on_wait=excess[i : i + _MAX_WAITS], on_update=[]
                        )
                        out.append(nop)
                    si.on_wait = keep
                out.append(inst)
            if changed:
                bb.instructions = out
    return ctr

def _elide_redundant_ldweights(nc):
    """Delete an InstLdweights that reloads the exact weights AP loaded by
    the previous (surviving) InstLdweights when only plain matmuls / no-ops
    sit between them in the scheduled stream.  The PE array keeps the
    stationary operand across matmuls, so the reload is pure overhead
    (walrus emits one LDWEIGHTS per MATMUL and its ldw-opt pass is
    incompatible with pre-split LDW+MM).  Only sync-free LDWs are removed,
    so semaphore bookkeeping is unchanged."""
    n_elided = 0
    for f in nc.m.functions:
        for bb in f.blocks:
            out = []
            last_w = None  # weights-AP repr of last surviving LDW, if run intact
            changed = False
            for inst in bb.instructions:
                nm = type(inst).__name__
                if nm == "InstLdweights":
                    si = inst.sync_info
                    clean = not (si and (si.on_wait or si.on_update))
                    w = repr(inst.ins[0])
                    if clean and last_w == w:
                        n_elided += 1
                        changed = True
                        continue  # drop the reload
                    last_w = w if clean else None
                elif nm == "InstMatmult":
                    if getattr(inst, "is_transpose", False):
                        last_w = None
                elif nm == "InstNoOp":
                    pass
                else:
                    last_w = None
                out.append(inst)
            if changed:
                bb.instructions = out
    return n_elided


NCORES = 8
N = 16384
SH = N // NCORES  # 2048 adj rows per core
F = 512
HID = 1024
OUT = 256

BF16 = mybir.dt.bfloat16
F32 = mybir.dt.float32

_built = None


def build():
    """Build the per-core Bass program (identical on all cores)."""
    nc = bass.Bass()

    adjT = nc.declare_dram_parameter("adjT", [N, SH], BF16, isOutput=False)
    xT = nc.declare_dram_parameter("xT", [F, SH], BF16, isOutput=False)
    w1 = nc.declare_dram_parameter("w1", [F, HID], BF16, isOutput=False)
    w2 = nc.declare_dram_parameter("w2", [HID, OUT], BF16, isOutput=False)
    b1T = nc.declare_dram_parameter("b1T", [128, HID // 128], F32, isOutput=False)
    b2T = nc.declare_dram_parameter("b2T", [128, OUT // 128], F32, isOutput=False)
    out2T = nc.declare_dram_parameter("out2T", [OUT, SH], F32, isOutput=True)

    rg = [list(range(NCORES))]

    # adjT column-block mb (512 wide), 4 k-blocks per DMA:
    #   [p, k4, kk, m] = adjT[k4*512 + kk*128 + p, mb*512 + m]
    def adjT_src(mb):
        return adjT[:, mb * 512 : (mb + 1) * 512].rearrange(
            "(k4 kk p) m -> p k4 kk m", kk=4, p=128
        )

    def adjTp_src(mbp):
        return adjT[:, mbp * 1024 : (mbp + 1) * 1024].rearrange(
            "(k4 kk p) m -> p k4 kk m", kk=4, p=128
        )

    def allgather(inp, outp):
        return nc.gpsimd.collective_compute(
            "AllGather",
            mybir.AluOpType.bypass,
            replica_groups=rg,
            ins=[inp.opt()],
            outs=[outp.opt()],
        )

    with tile.TileContext(nc) as tc:
        with (
            tc.tile_pool(name="const", bufs=1) as constp,
            tc.tile_pool(name="psum", bufs=8, space="PSUM") as psum,
            tc.tile_pool(name="dram", bufs=1, space="DRAM") as dram,
            tc.tile_pool(name="adj", bufs=4) as adjp,
            tc.tile_pool(name="small", bufs=4) as smallp,
        ):
            # ---- constants ----
            w2t = constp.tile([128, HID // 128, OUT], BF16)
            nc.sync.dma_start(w2t[:], w2[:].rearrange("(f p) n -> p f n", p=128))
            b1t = constp.tile([128, HID // 128], F32)
            nc.sync.dma_start(b1t[:], b1T[:])
            b2t = constp.tile([128, OUT // 128], F32)
            nc.sync.dma_start(b2t[:], b2T[:])

            # AllGathers split in quarters so they overlap compute: phase B
            # can start once the first two s1 quarters have gathered, and
            # phase D streams k-blocks in gather-arrival order.
            ag1_in = [dram.tile([SH, 512], BF16, name=f"ag1i{h}") for h in range(2)]
            ag1_out = [
                dram.tile([N, 512], BF16, addr_space="Shared", name=f"ag1o{h}")
                for h in range(2)
            ]
            ag2_in = [dram.tile([SH // 4, OUT], BF16, name=f"ag2i{q}") for q in range(4)]
            ag2_out = [
                dram.tile([N // 4, OUT], BF16, addr_space="Shared", name=f"ag2o{q}")
                for q in range(4)
            ]

            # ---- phase A: s1_c = x_c @ W1 (per n-quarter; AG per quarter) ----
            with tc.tile_pool(name="phA", bufs=1) as pA:
                xt = []
                w1t = []
                for f in range(4):
                    t = pA.tile([128, SH], BF16, name=f"xt{f}")
                    nc.sync.dma_start(t[:], xT[f * 128 : (f + 1) * 128, :])
                    xt.append(t)
                    t = pA.tile([128, HID], BF16, name=f"w1t{f}")
                    nc.sync.dma_start(t[:], w1[f * 128 : (f + 1) * 128, :])
                    w1t.append(t)
                # half 0 gathers immediately (it gates phase B's start);
                # half 1 is computed now but gathered later (delayed dep)
                for h in range(2):
                    for mt in range(SH // 128):
                        psa = psum.tile([128, 512], F32, tag="ps", name=f"psA{h}{mt}")
                        for f in range(4):
                            nc.tensor.matmul(
                                psa[:],
                                xt[f][:, mt * 128 : (mt + 1) * 128],
                                w1t[f][:, h * 512 : (h + 1) * 512],
                                start=(f == 0),
                                stop=(f == 3),
                            )
                        s1o = smallp.tile([128, 512], BF16, tag="s1o", bufs=2)
                        nc.vector.tensor_copy(s1o[:], psa[:])
                        nc.scalar.dma_start(
                            ag1_in[h][mt * 128 : (mt + 1) * 128, :], s1o[:]
                        )
                    if h == 0:
                        allgather(ag1_in[0], ag1_out[0])

            # ---- phases B + C (C quarters interleaved so AG2 fires early) --
            with (
                tc.tile_pool(name="s1res", bufs=32) as s1p,
                tc.tile_pool(name="ht", bufs=32) as htp,
            ):
                ht_tiles = {}

                def phase_c_quarter(qq):
                    # s2 rows qq*512 .. +511 (needs ht tiles mb=qq, all f)
                    for mth in range(4):
                        mt = qq * 4 + mth
                        mb, off = mt // 4, (mt % 4) * 128
                        psc = psum.tile([128, OUT], F32, tag="ps", name=f"psC{mt}")
                        for f in range(8):
                            nc.tensor.matmul(
                                psc[:],
                                ht_tiles[(f, mb)][:, off : off + 128],
                                w2t[:, f, :],
                                start=(f == 0),
                                stop=(f == 7),
                            )
                        s2o = smallp.tile([128, OUT], BF16, tag="s2o", bufs=2)
                        nc.vector.tensor_copy(s2o[:], psc[:])
                        nc.scalar.dma_start(
                            ag2_in[qq][mth * 128 : (mth + 1) * 128, :], s2o[:]
                        )
                    allgather(ag2_in[qq], ag2_out[qq])

                for nh in range(2):
                    s1_src = ag1_out[nh][:].rearrange(
                        "(k4 kk p) n -> p k4 kk n", kk=4, p=128
                    )
                    s1t = []
                    # m-blocks processed in pairs: each stationary s1 slice
                    # feeds 2 matmuls (adjacent mb), halving LDWEIGHTS count.
                    for mbp in range(2):
                        ps = [
                            psum.tile(
                                [128, 512], F32, tag="ps", name=f"psB{nh}{mbp}{i}"
                            )
                            for i in range(8)
                        ]  # index nt*2 + mbx
                        for k4 in range(32):
                            if mbp == 0:
                                t = s1p.tile(
                                    [128, 4, 512], BF16, tag="s1t",
                                    name=f"s1t{nh}{k4}",
                                )
                                nc.sync.dma_start(t[:], s1_src[:, k4])
                                s1t.append(t)
                            ats = []
                            for mbx in range(2):
                                atx = adjp.tile(
                                    [128, 4, 512], BF16, tag="adjt", bufs=4,
                                    name=f"at{nh}{mbp}{k4}{mbx}",
                                )
                                nc.sync.dma_start(
                                    atx[:], adjT_src(mbp * 2 + mbx)[:, k4]
                                )
                                ats.append(atx)
                            for kk in range(4):
                                k = k4 * 4 + kk
                                for nt in range(4):
                                    lhs = s1t[k4][:, kk, nt * 128 : (nt + 1) * 128]
                                    for mbx in range(2):
                                        nc.tensor.matmul(
                                            ps[nt * 2 + mbx][:],
                                            lhs,
                                            ats[mbx][:, kk, :],
                                            start=(k == 0),
                                            stop=(k == 127),
                                        )
                        last_act = None
                        for nt in range(4):
                            j = nh * 4 + nt
                            for mbx in range(2):
                                mb = mbp * 2 + mbx
                                htt = htp.tile([128, 512], BF16, tag="htt")
                                last_act = nc.scalar.activation(
                                    htt[:],
                                    ps[nt * 2 + mbx][:],
                                    mybir.ActivationFunctionType.Relu,
                                    bias=b1t[:, j : j + 1],
                                )
                                ht_tiles[(j, mb)] = htt
                        if nh == 0 and mbp == 0:
                            # fire the second-half s1 gather now; dep delays
                            # its SDMA traffic past B's startup loads
                            cc = allgather(ag1_in[1], ag1_out[1])
                            tile_rust_add_dep(
                                cc.ins,
                                last_act.ins,
                                sync=True,
                                reason="delay s1 half-1 gather past B start",
                            )
                        if nh == 1:
                            # ht tiles for mb 2*mbp..2*mbp+1 now complete for
                            # all f -> emit the matching C quarters + gathers.
                            phase_c_quarter(2 * mbp)
                            phase_c_quarter(2 * mbp + 1)

            # ---- phase D: out2T = (adj_c @ s2)^T + b2 ----
            # All 8 psum banks accumulate concurrently; k-blocks consumed in
            # gather-arrival order (quarter-major), s2 tiles loaded JIT after
            # each adjT chunk so the SP queue stays load-ordered.
            with (
                tc.tile_pool(name="s2res", bufs=32) as s2p,
                tc.tile_pool(name="adjD", bufs=4) as adjDp,
                tc.tile_pool(name="outp", bufs=8) as outp,
            ):
                # ag2_out[qq] rows = g*512 + skk*128 + p  (rank g, block qq)
                s2_srcs = [
                    ag2_out[qq][:].rearrange("(g skk p) n -> p g skk n", g=8, p=128)
                    for qq in range(4)
                ]
                adjD_src = adjT[:].rearrange("(k4 kk p) m -> p k4 kk m", kk=4, p=128)
                dps = [
                    psum.tile([128, 512], F32, tag="ps", name=f"psD{i}")
                    for i in range(8)
                ]
                # k4 = g*4 + qq  ->  iterate quarter-major
                k4_order = [g * 4 + qq for qq in range(4) for g in range(8)]
                for ki, k4 in enumerate(k4_order):
                    g, qq = k4 // 4, k4 % 4
                    at = adjDp.tile([128, 4, SH], BF16, tag="adjD", name=f"atD{k4}")
                    nc.sync.dma_start(at[:], adjD_src[:, k4])
                    st = s2p.tile([128, 4, OUT], BF16, tag="s2t", name=f"s2t{k4}")
                    nc.sync.dma_start(st[:], s2_srcs[qq][:, g])
                    for kk in range(4):
                        for n2t in range(2):
                            lhs = st[:, kk, n2t * 128 : (n2t + 1) * 128]
                            for mb in range(4):
                                nc.tensor.matmul(
                                    dps[n2t * 4 + mb][:],
                                    lhs,
                                    at[:, kk, mb * 512 : (mb + 1) * 512],
                                    start=(ki == 0 and kk == 0),
                                    stop=(ki == 31 and kk == 3),
                                )
                for n2t in range(2):
                    for mb in range(4):
                        ot = outp.tile([128, 512], F32, tag="ot")
                        nc.scalar.activation(
                            ot[:],
                            dps[n2t * 4 + mb][:],
                            mybir.ActivationFunctionType.Identity,
                            bias=b2t[:, n2t : n2t + 1],
                        )
                        nc.scalar.dma_start(
                            out2T[
                                n2t * 128 : (n2t + 1) * 128, mb * 512 : (mb + 1) * 512
                            ],
                            ot[:],
                        )

    _elide_redundant_ldweights(nc)
    _split_excess_waits(nc)
    return nc


def _prep_inputs(x, adj, W1, b1, W2, b2):
    bf = ml_dtypes.bfloat16
    w1b = W1.astype(bf)
    w2b = W2.astype(bf)
    b1T = np.ascontiguousarray(b1.reshape(HID // 128, 128).T).astype(np.float32)
    b2T = np.ascontiguousarray(b2.reshape(OUT // 128, 128).T).astype(np.float32)
    in_maps = []
    for c in range(NCORES):
        rows = slice(c * SH, (c + 1) * SH)
        in_maps.append(
            {
                "adjT": adj[rows, :].T.astype(bf),
                "xT": x[rows, :].T.astype(bf),
                "w1": w1b,
                "w2": w2b,
                "b1T": b1T,
                "b2T": b2T,
            }
        )
    return in_maps


def _run(inputs, trace=False):
    global _built
    if _built is None:
        _built = build()
    in_maps = _prep_inputs(**inputs)
    r = run_bass_kernel_spmd(_built, in_maps, list(range(NCORES)), trace=trace)
    out = np.empty([N, OUT], np.float32)
    for c in range(NCORES):
        out[c * SH : (c + 1) * SH, :] = r.results[c]["out2T"].T
    return out, r


def kernel(x, adj, W1, b1, W2, b2):
    out, _ = _run(dict(x=x, adj=adj, W1=W1, b1=b1, W2=W2, b2=b2))
    return out



# revision 2
# speedup vs baseline: 1.9194x; 1.9194x over previous
"""Trainium2 Bass kernel for a 2-layer dense GCN (NodeEncoder).

    out = adj @ relu(adj @ (x@W1) + b1) @ W2 + b2
    N=16384, F_IN=512, HID=1024, OUT=256, adj dense [N, N] fp32.

Sharding: adj row-partitioned across 8 NeuronCores (2048 rows/core).
All device matmuls consume natural-layout (row-major) operands; the
host pre-transposes adj/x per shard so no on-device transposes are
needed.  Per core:

  phase A:  s1_c   = x_c @ W1                  [2048, 1024]  (own rows)
  AG1:      s1     = AllGather(s1_c)           [16384, 1024]
  phase B:  hT_c   = relu(adj_c @ s1 + b1)^T   [1024, 2048]  (transposed
            orientation: lhsT = s1 tiles, rhs = adjT_c tiles -> psum is
            [n, m]; bias b1 is per-partition, fused into the ACT relu)
  phase C:  s2_c   = h_c @ W2                  [2048, 256]   (lhsT = hT_c)
  AG2:      s2     = AllGather(s2_c)           [16384, 256]
  phase D:  out2T_c = (adj_c @ s2)^T + b2      [256, 2048]   (lhsT = s2
            tiles, rhs = adjT_c tiles; b2 per-partition via ACT)

The two adj-sized matmuls (B, D: 94% of FLOPs) run in fp8e4 (e4m3,
max 240) with MatmulPerfMode.DoubleRow: both operands fp8, K=256 per
matmul, 2x TensorE throughput vs bf16.  adj is pre-scaled by 2^21 on
the host (values in [0,128]); s1/s2 are quantized on-device with
power-of-2 scales; the combined scale is divided out in the psum->SBUF
activation (which also applies bias/relu).  A and C stay bf16.
Host-simulated rel err vs the fp32 reference: 1.86e-2 of absmax
(gate 2e-2); bf16 everywhere gives 2.87e-3.
"""

import numpy as np
import ml_dtypes

import concourse.bass as bass
import concourse.mybir as mybir
import concourse.tile as tile
from concourse.bass_utils import run_bass_kernel_spmd
from concourse.tile_sem_assignment import N_PROCS
from concourse.vector_clock import ScopedClock, VectorClock
from concourse.tile_rust import add_dep_helper as tile_rust_add_dep

# ---------------------------------------------------------------------------
# Workaround: the walrus build in this container caps the number of sync-wait
# commands on a Drain instruction; Tile's kernel-tail drain aggregates one
# wait per logical processor and exceeds it.  Split the tail drain into a
# chain of single-wait drains on the same (SP) queue — semantically identical.
# ---------------------------------------------------------------------------


def _drain_and_barrier_split(self, tick_clock, wait_clock):
    gc = tick_clock.global_clock
    for p in range(N_PROCS):
        partial = VectorClock([gc[q] if q == p else 0 for q in range(N_PROCS)])
        d = self.nc.sync.drain()
        wait_clock.add_sem_waits(d.ins, ScopedClock({None: partial}))
    self.nc.sync.drain()

    self.nc.all_engine_barrier()
    assert self.sems is not None
    popped = self.nc._tile_sem_poison_stack.pop()
    assert popped is self._sem_poison
    self.nc.clear_and_free_semaphores(list(self.sems.allocated().values()))
    self.nc.all_engine_barrier()


tile.TileContext._drain_and_barrier = _drain_and_barrier_split

# The same walrus cap applies to every instruction kind: at most ONE sync
# wait command per instruction (probed empirically — a 2-wait TensorCopy is
# rejected).  Post-pass: hoist excess sem-waits onto no-ops inserted just
# before the instruction on the same engine queue — per-engine program order
# makes this semantically identical.
_MAX_WAITS = 1


def _split_excess_waits(nc):
    ctr = 0
    for f in nc.m.functions:
        for bb in f.blocks:
            out = []
            changed = False
            for inst in bb.instructions:
                si = inst.sync_info
                waits = list(si.on_wait) if si is not None and si.on_wait else []
                if len(waits) > _MAX_WAITS:
                    changed = True
                    keep, excess = waits[: _MAX_WAITS], waits[_MAX_WAITS :]
                    for i in range(0, len(excess), _MAX_WAITS):
                        ctr += 1
                        nop = mybir.InstNoOp(name=f"I-waitnop-{ctr}")
                        nop.engine = inst.engine
                        nop.sync_info = mybir.SyncInfo(
                            on_wait=excess[i : i + _MAX_WAITS], on_update=[]
                        )
                        out.append(nop)
                    si.on_wait = keep
                out.append(inst)
            if changed:
                bb.instructions = out
    return ctr


def _elide_redundant_ldweights(nc):
    """Delete an InstLdweights that reloads the exact weights AP loaded by
    the previous (surviving) InstLdweights when only plain matmuls / no-ops
    sit between them in the scheduled stream.  The PE array keeps the
    stationary operand across matmuls, so the reload is pure overhead
    (walrus emits one LDWEIGHTS per MATMUL and its ldw-opt pass is
    incompatible with pre-split LDW+MM).  Only sync-free LDWs are removed,
    so semaphore bookkeeping is unchanged."""
    n_elided = 0
    for f in nc.m.functions:
        for bb in f.blocks:
            out = []
            last_w = None  # weights-AP repr of last surviving LDW, if run intact
            changed = False
            for inst in bb.instructions:
                nm = type(inst).__name__
                if nm == "InstLdweights":
                    si = inst.sync_info
                    clean = not (si and (si.on_wait or si.on_update))
                    w = repr(inst.ins[0])
                    if clean and last_w == w:
                        n_elided += 1
                        changed = True
                        continue  # drop the reload
                    last_w = w if clean else None
                elif nm == "InstMatmult":
                    if getattr(inst, "is_transpose", False):
                        last_w = None
                elif nm == "InstNoOp":
                    pass
                else:
                    last_w = None
                out.append(inst)
            if changed:
                bb.instructions = out
    return n_elided


NCORES = 8
N = 16384
SH = N // NCORES  # 2048 adj rows per core
F = 512
HID = 1024
OUT = 256

BF16 = mybir.dt.bfloat16
F8 = mybir.dt.float8e4
F32 = mybir.dt.float32
NPF8 = ml_dtypes.float8_e4m3
DR = mybir.MatmulPerfMode.DoubleRow

# fp8 scale plan: adj pre-scaled on host; s1/s2 scaled into fp8 on device;
# the product scale is divided out in the psum-draining activation.
S_ADJ = 2.0**21  # adj in [0, 1/N]   -> [0, 128]
S_S1 = 2.0**4  # s1 absmax ~6.3    -> ~101
S_S2 = 2.0**13  # s2 absmax ~0.017  -> ~135
B_DESCALE = 1.0 / (S_ADJ * S_S1)  # 2^-25
D_DESCALE = 1.0 / (S_ADJ * S_S2)  # 2^-34

_built = None


def build():
    """Build the per-core Bass program (identical on all cores)."""
    nc = bass.Bass()

    adjT = nc.declare_dram_parameter("adjT", [N, SH], F8, isOutput=False)
    xT = nc.declare_dram_parameter("xT", [F, SH], BF16, isOutput=False)
    w1 = nc.declare_dram_parameter("w1", [F, HID], BF16, isOutput=False)
    w2 = nc.declare_dram_parameter("w2", [HID, OUT], BF16, isOutput=False)
    b1T = nc.declare_dram_parameter("b1T", [128, HID // 128], F32, isOutput=False)
    b2T = nc.declare_dram_parameter("b2T", [128, OUT // 128], F32, isOutput=False)
    out2T = nc.declare_dram_parameter("out2T", [OUT, SH], F32, isOutput=True)

    rg = [list(range(NCORES))]

    # adjT column-block pair mbp (1024 wide), 4 k-blocks per DMA, 1KB lines:
    #   [p, k4, kk, m] = adjT[k4*512 + kk*128 + p, mbp*1024 + m]
    def adjTp_src(mbp):
        return adjT[:, mbp * 1024 : (mbp + 1) * 1024].rearrange(
            "(k4 kk p) m -> p k4 kk m", kk=4, p=128
        )

    def allgather(inp, outp):
        return nc.gpsimd.collective_compute(
            "AllGather",
            mybir.AluOpType.bypass,
            replica_groups=rg,
            ins=[inp.opt()],
            outs=[outp.opt()],
        )

    with tile.TileContext(nc) as tc:
        with (
            tc.tile_pool(name="const", bufs=1) as constp,
            tc.tile_pool(name="psum", bufs=8, space="PSUM") as psum,
            tc.tile_pool(name="dram", bufs=1, space="DRAM") as dram,
            tc.tile_pool(name="adj", bufs=4) as adjp,
            tc.tile_pool(name="small", bufs=4) as smallp,
        ):
            # ---- constants ----
            w2t = constp.tile([128, HID // 128, OUT], BF16)
            nc.sync.dma_start(w2t[:], w2[:].rearrange("(f p) n -> p f n", p=128))
            b1t = constp.tile([128, HID // 128], F32)
            nc.sync.dma_start(b1t[:], b1T[:])
            b2t = constp.tile([128, OUT // 128], F32)
            nc.sync.dma_start(b2t[:], b2T[:])

            # AllGathers split in quarters so they overlap compute: phase B
            # can start once the first two s1 quarters have gathered, and
            # phase D streams k-blocks in gather-arrival order.
            ag1_in = [dram.tile([SH, 512], F8, name=f"ag1i{h}") for h in range(2)]
            ag1_out = [
                dram.tile([N, 512], F8, addr_space="Shared", name=f"ag1o{h}")
                for h in range(2)
            ]
            ag2_in = [dram.tile([SH // 4, OUT], F8, name=f"ag2i{q}") for q in range(4)]
            ag2_out = [
                dram.tile([N // 4, OUT], F8, addr_space="Shared", name=f"ag2o{q}")
                for q in range(4)
            ]

            # ---- phase A: s1_c = x_c @ W1 (per n-half; AG per half) ----
            with tc.tile_pool(name="phA", bufs=1) as pA:
                xt = []
                w1t = []
                for f in range(4):
                    t = pA.tile([128, SH], BF16, name=f"xt{f}")
                    nc.sync.dma_start(t[:], xT[f * 128 : (f + 1) * 128, :])
                    xt.append(t)
                    t = pA.tile([128, HID], BF16, name=f"w1t{f}")
                    nc.sync.dma_start(t[:], w1[f * 128 : (f + 1) * 128, :])
                    w1t.append(t)
                # half 0 gathers immediately (it gates phase B's start);
                # half 1 is computed now but gathered later (delayed dep)
                for h in range(2):
                    for mt in range(SH // 128):
                        psa = psum.tile([128, 512], F32, tag="ps", name=f"psA{h}{mt}")
                        for f in range(4):
                            nc.tensor.matmul(
                                psa[:],
                                xt[f][:, mt * 128 : (mt + 1) * 128],
                                w1t[f][:, h * 512 : (h + 1) * 512],
                                start=(f == 0),
                                stop=(f == 3),
                            )
                        s1o = smallp.tile([128, 512], F8, tag="s1o", bufs=2)
                        nc.scalar.activation(
                            s1o[:],
                            psa[:],
                            mybir.ActivationFunctionType.Copy,
                            scale=S_S1,
                        )
                        nc.scalar.dma_start(
                            ag1_in[h][mt * 128 : (mt + 1) * 128, :], s1o[:]
                        )
                    if h == 0:
                        allgather(ag1_in[0], ag1_out[0])

            # ---- phases B + C (C quarters interleaved so AG2 fires early) --
            with (
                tc.tile_pool(name="s1res", bufs=32) as s1p,
                tc.tile_pool(name="ht", bufs=32) as htp,
            ):
                ht_tiles = {}

                def phase_c_quarter(qq):
                    # s2 rows qq*512 .. +511 (needs ht tiles mb=qq, all f)
                    for mth in range(4):
                        mt = qq * 4 + mth
                        mb, off = mt // 4, (mt % 4) * 128
                        psc = psum.tile([128, OUT], F32, tag="ps", name=f"psC{mt}")
                        for f in range(8):
                            nc.tensor.matmul(
                                psc[:],
                                ht_tiles[(f, mb)][:, off : off + 128],
                                w2t[:, f, :],
                                start=(f == 0),
                                stop=(f == 7),
                            )
                        s2o = smallp.tile([128, OUT], F8, tag="s2o", bufs=2)
                        nc.scalar.activation(
                            s2o[:],
                            psc[:],
                            mybir.ActivationFunctionType.Copy,
                            scale=S_S2,
                        )
                        nc.scalar.dma_start(
                            ag2_in[qq][mth * 128 : (mth + 1) * 128, :], s2o[:]
                        )
                    allgather(ag2_in[qq], ag2_out[qq])

                for nh in range(2):
                    s1_src = ag1_out[nh][:].rearrange(
                        "(k4 kk p) n -> p k4 kk n", kk=4, p=128
                    )
                    s1t = []
                    for mbp in range(2):
                        ps = [
                            psum.tile(
                                [128, 512], F32, tag="ps", name=f"psB{nh}{mbp}{i}"
                            )
                            for i in range(8)
                        ]  # index nt*2 + mbx
                        for k4 in range(32):
                            if mbp == 0:
                                t = s1p.tile(
                                    [128, 4, 512], F8, tag="s1t",
                                    name=f"s1t{nh}{k4}",
                                )
                                nc.sync.dma_start(t[:], s1_src[:, k4])
                                s1t.append(t)
                            atp = adjp.tile(
                                [128, 4, 1024], F8, tag="adjt", bufs=4,
                                name=f"at{nh}{mbp}{k4}",
                            )
                            nc.sync.dma_start(atp[:], adjTp_src(mbp)[:, k4])
                            for kkp in range(2):
                                for nt in range(4):
                                    lhs = s1t[k4][
                                        :, 2 * kkp : 2 * kkp + 2,
                                        nt * 128 : (nt + 1) * 128,
                                    ]
                                    for mbx in range(2):
                                        nc.tensor.matmul(
                                            ps[nt * 2 + mbx][:],
                                            lhs,
                                            atp[
                                                :, 2 * kkp : 2 * kkp + 2,
                                                mbx * 512 : (mbx + 1) * 512,
                                            ],
                                            start=(k4 == 0 and kkp == 0),
                                            stop=(k4 == 31 and kkp == 1),
                                            perf_mode=DR,
                                        )
                        last_act = None
                        for nt in range(4):
                            j = nh * 4 + nt
                            for mbx in range(2):
                                mb = mbp * 2 + mbx
                                htt = htp.tile([128, 512], BF16, tag="htt")
                                last_act = nc.scalar.activation(
                                    htt[:],
                                    ps[nt * 2 + mbx][:],
                                    mybir.ActivationFunctionType.Relu,
                                    bias=b1t[:, j : j + 1],
                                    scale=B_DESCALE,
                                )
                                ht_tiles[(j, mb)] = htt
                        if nh == 0 and mbp == 0:
                            # fire the second-half s1 gather now; dep delays
                            # its SDMA traffic past B's startup loads
                            cc = allgather(ag1_in[1], ag1_out[1])
                            tile_rust_add_dep(
                                cc.ins,
                                last_act.ins,
                                sync=True,
                                reason="delay s1 half-1 gather past B start",
                            )
                        if nh == 1:
                            # ht tiles for mb 2*mbp..2*mbp+1 now complete for
                            # all f -> emit the matching C quarters + gathers.
                            phase_c_quarter(2 * mbp)
                            phase_c_quarter(2 * mbp + 1)

            # ---- phase D: out2T = (adj_c @ s2)^T + b2 ----
            # All 8 psum banks accumulate concurrently; k-blocks consumed in
            # gather-arrival order (quarter-major), s2 tiles loaded JIT after
            # each adjT chunk so the SP queue stays load-ordered.
            with (
                tc.tile_pool(name="s2res", bufs=32) as s2p,
                tc.tile_pool(name="adjD", bufs=4) as adjDp,
                tc.tile_pool(name="outp", bufs=8) as outp,
            ):
                # ag2_out[qq] rows = g*512 + skk*128 + p  (rank g, block qq)
                s2_srcs = [
                    ag2_out[qq][:].rearrange("(g skk p) n -> p g skk n", g=8, p=128)
                    for qq in range(4)
                ]
                adjD_src = adjT[:].rearrange("(k4 kk p) m -> p k4 kk m", kk=4, p=128)
                dps = [
                    psum.tile([128, 512], F32, tag="ps", name=f"psD{i}")
                    for i in range(8)
                ]
                # k4 = g*4 + qq  ->  iterate quarter-major
                k4_order = [g * 4 + qq for qq in range(4) for g in range(8)]
                for ki, k4 in enumerate(k4_order):
                    g, qq = k4 // 4, k4 % 4
                    at = adjDp.tile([128, 4, SH], F8, tag="adjD", name=f"atD{k4}")
                    nc.sync.dma_start(at[:], adjD_src[:, k4])
                    st = s2p.tile([128, 4, OUT], F8, tag="s2t", name=f"s2t{k4}")
                    nc.sync.dma_start(st[:], s2_srcs[qq][:, g])
                    for kkp in range(2):
                        for n2t in range(2):
                            lhs = st[
                                :, 2 * kkp : 2 * kkp + 2,
                                n2t * 128 : (n2t + 1) * 128,
                            ]
                            for mb in range(4):
                                nc.tensor.matmul(
                                    dps[n2t * 4 + mb][:],
                                    lhs,
                                    at[
                                        :, 2 * kkp : 2 * kkp + 2,
                                        mb * 512 : (mb + 1) * 512,
                                    ],
                                    start=(ki == 0 and kkp == 0),
                                    stop=(ki == 31 and kkp == 1),
                                    perf_mode=DR,
                                )
                for n2t in range(2):
                    for mb in range(4):
                        ot = outp.tile([128, 512], F32, tag="ot")
                        nc.scalar.activation(
                            ot[:],
                            dps[n2t * 4 + mb][:],
                            mybir.ActivationFunctionType.Identity,
                            bias=b2t[:, n2t : n2t + 1],
                            scale=D_DESCALE,
                        )
                        nc.scalar.dma_start(
                            out2T[
                                n2t * 128 : (n2t + 1) * 128, mb * 512 : (mb + 1) * 512
                            ],
                            ot[:],
                        )

    _elide_redundant_ldweights(nc)
    _split_excess_waits(nc)
    return nc


def _prep_inputs(x, adj, W1, b1, W2, b2):
    bf = ml_dtypes.bfloat16
    w1b = W1.astype(bf)
    w2b = W2.astype(bf)
    b1T = np.ascontiguousarray(b1.reshape(HID // 128, 128).T).astype(np.float32)
    b2T = np.ascontiguousarray(b2.reshape(OUT // 128, 128).T).astype(np.float32)
    in_maps = []
    for c in range(NCORES):
        rows = slice(c * SH, (c + 1) * SH)
        adj8 = np.clip(adj[rows, :].T * np.float32(S_ADJ), -240.0, 240.0).astype(NPF8)
        in_maps.append(
            {
                "adjT": adj8,
                "xT": x[rows, :].T.astype(bf),
                "w1": w1b,
                "w2": w2b,
                "b1T": b1T,
                "b2T": b2T,
            }
        )
    return in_maps


def _run(inputs, trace=False):
    global _built
    if _built is None:
        _built = build()
    in_maps = _prep_inputs(**inputs)
    r = run_bass_kernel_spmd(_built, in_maps, list(range(NCORES)), trace=trace)
    out = np.empty([N, OUT], np.float32)
    for c in range(NCORES):
        out[c * SH : (c + 1) * SH, :] = r.results[c]["out2T"].T
    return out, r


def kernel(x, adj, W1, b1, W2, b2):
    out, _ = _run(dict(x=x, adj=adj, W1=W1, b1=b1, W2=W2, b2=b2))
    return out


# revision 7
# speedup vs baseline: 1.9619x; 1.0222x over previous
"""Trainium2 Bass kernel for a 2-layer dense GCN (NodeEncoder).

    out = adj @ relu(adj @ (x@W1) + b1) @ W2 + b2
    N=16384, F_IN=512, HID=1024, OUT=256, adj dense [N, N] fp32.

Sharding: adj row-partitioned across 8 NeuronCores (2048 rows/core).
All device matmuls consume natural-layout (row-major) operands; the
host pre-transposes adj/x per shard so no on-device transposes are
needed.  Per core:

  phase A:  s1_c   = x_c @ W1                  [2048, 1024]  (own rows)
  AG1:      s1     = AllGather(s1_c)           [16384, 1024]
  phase B:  hT_c   = relu(adj_c @ s1 + b1)^T   [1024, 2048]  (transposed
            orientation: lhsT = s1 tiles, rhs = adjT_c tiles -> psum is
            [n, m]; bias b1 is per-partition, fused into the ACT relu)
  phase C:  s2_c   = h_c @ W2                  [2048, 256]   (lhsT = hT_c)
  AG2:      s2     = AllGather(s2_c)           [16384, 256]
  phase D:  out2T_c = (adj_c @ s2)^T + b2      [256, 2048]   (lhsT = s2
            tiles, rhs = adjT_c tiles; b2 per-partition via ACT)

The two adj-sized matmuls (B, D: 94% of FLOPs) run in fp8e4 (e4m3,
max 240) with MatmulPerfMode.DoubleRow: both operands fp8, K=256 per
matmul, 2x TensorE throughput vs bf16.  adj is pre-scaled by 2^21 on
the host (values in [0,128]); s1/s2 are quantized on-device with
power-of-2 scales; the combined scale is divided out in the psum->SBUF
activation (which also applies bias/relu).  A and C stay bf16.
Host-simulated rel err vs the fp32 reference: 1.86e-2 of absmax
(gate 2e-2); bf16 everywhere gives 2.87e-3.
"""

import numpy as np
import ml_dtypes

import concourse.bass as bass
import concourse.mybir as mybir
import concourse.tile as tile
from concourse.bass_utils import run_bass_kernel_spmd
from concourse.tile_sem_assignment import N_PROCS
from concourse.vector_clock import ScopedClock, VectorClock
from concourse.tile_rust import add_dep_helper as tile_rust_add_dep

# ---------------------------------------------------------------------------
# Workaround: the walrus build in this container caps the number of sync-wait
# commands on a Drain instruction; Tile's kernel-tail drain aggregates one
# wait per logical processor and exceeds it.  Split the tail drain into a
# chain of single-wait drains on the same (SP) queue — semantically identical.
# ---------------------------------------------------------------------------


def _drain_and_barrier_split(self, tick_clock, wait_clock):
    gc = tick_clock.global_clock
    for p in range(N_PROCS):
        partial = VectorClock([gc[q] if q == p else 0 for q in range(N_PROCS)])
        d = self.nc.sync.drain()
        wait_clock.add_sem_waits(d.ins, ScopedClock({None: partial}))
    self.nc.sync.drain()

    self.nc.all_engine_barrier()
    assert self.sems is not None
    popped = self.nc._tile_sem_poison_stack.pop()
    assert popped is self._sem_poison
    self.nc.clear_and_free_semaphores(list(self.sems.allocated().values()))
    self.nc.all_engine_barrier()


tile.TileContext._drain_and_barrier = _drain_and_barrier_split

# The same walrus cap applies to every instruction kind: at most ONE sync
# wait command per instruction (probed empirically — a 2-wait TensorCopy is
# rejected).  Post-pass: hoist excess sem-waits onto no-ops inserted just
# before the instruction on the same engine queue — per-engine program order
# makes this semantically identical.
_MAX_WAITS = 1


def _split_excess_waits(nc):
    ctr = 0
    for f in nc.m.functions:
        for bb in f.blocks:
            out = []
            changed = False
            for inst in bb.instructions:
                si = inst.sync_info
                waits = list(si.on_wait) if si is not None and si.on_wait else []
                if len(waits) > _MAX_WAITS:
                    changed = True
                    keep, excess = waits[: _MAX_WAITS], waits[_MAX_WAITS :]
                    for i in range(0, len(excess), _MAX_WAITS):
                        ctr += 1
                        nop = mybir.InstNoOp(name=f"I-waitnop-{ctr}")
                        nop.engine = inst.engine
                        nop.sync_info = mybir.SyncInfo(
                            on_wait=excess[i : i + _MAX_WAITS], on_update=[]
                        )
                        out.append(nop)
                    si.on_wait = keep
                out.append(inst)
            if changed:
                bb.instructions = out
    return ctr


def _elide_redundant_ldweights(nc):
    """Delete an InstLdweights that reloads the exact weights AP loaded by
    the previous (surviving) InstLdweights when only plain matmuls / no-ops
    sit between them in the scheduled stream.  The PE array keeps the
    stationary operand across matmuls, so the reload is pure overhead
    (walrus emits one LDWEIGHTS per MATMUL and its ldw-opt pass is
    incompatible with pre-split LDW+MM).  Only sync-free LDWs are removed,
    so semaphore bookkeeping is unchanged."""
    n_elided = 0
    for f in nc.m.functions:
        for bb in f.blocks:
            out = []
            last_w = None  # weights-AP repr of last surviving LDW, if run intact
            changed = False
            for inst in bb.instructions:
                nm = type(inst).__name__
                if nm == "InstLdweights":
                    si = inst.sync_info
                    clean = not (si and (si.on_wait or si.on_update))
                    w = repr(inst.ins[0])
                    if clean and last_w == w:
                        n_elided += 1
                        changed = True
                        continue  # drop the reload
                    last_w = w if clean else None
                elif nm == "InstMatmult":
                    if getattr(inst, "is_transpose", False):
                        last_w = None
                elif nm == "InstNoOp":
                    pass
                else:
                    last_w = None
                out.append(inst)
            if changed:
                bb.instructions = out
    return n_elided


NCORES = 8
N = 16384
SH = N // NCORES  # 2048 adj rows per core
F = 512
HID = 1024
OUT = 256

BF16 = mybir.dt.bfloat16
F8 = mybir.dt.float8e4
F32 = mybir.dt.float32
NPF8 = ml_dtypes.float8_e4m3
DR = mybir.MatmulPerfMode.DoubleRow

# fp8 scale plan: adj pre-scaled on host; s1/s2 scaled into fp8 on device;
# the product scale is divided out in the psum-draining activation.
S_ADJ = 2.0**21  # adj in [0, 1/N]   -> [0, 128]
S_S1 = 2.0**4  # s1 absmax ~6.3    -> ~101
S_S2 = 2.0**13  # s2 absmax ~0.017  -> ~135
B_DESCALE = 1.0 / (S_ADJ * S_S1)  # 2^-25
D_DESCALE = 1.0 / (S_ADJ * S_S2)  # 2^-34

_built = None


def build():
    """Build the per-core Bass program (identical on all cores)."""
    nc = bass.Bass()

    adjT = nc.declare_dram_parameter("adjT", [N, SH], F8, isOutput=False)
    xT = nc.declare_dram_parameter("xT", [F, SH], BF16, isOutput=False)
    w1 = nc.declare_dram_parameter("w1", [F, HID], BF16, isOutput=False)
    w2 = nc.declare_dram_parameter("w2", [HID, OUT], BF16, isOutput=False)
    b1T = nc.declare_dram_parameter("b1T", [128, HID // 128], F32, isOutput=False)
    b2T = nc.declare_dram_parameter("b2T", [128, OUT // 128], F32, isOutput=False)
    out2T = nc.declare_dram_parameter("out2T", [OUT, SH], F32, isOutput=True)

    rg = [list(range(NCORES))]

    def allgather(inp, outp):
        return nc.gpsimd.collective_compute(
            "AllGather",
            mybir.AluOpType.bypass,
            replica_groups=rg,
            ins=[inp.opt()],
            outs=[outp.opt()],
        )

    with tile.TileContext(nc) as tc:
        with (
            tc.tile_pool(name="const", bufs=1) as constp,
            tc.tile_pool(name="psum", bufs=8, space="PSUM") as psum,
            tc.tile_pool(name="dram", bufs=1, space="DRAM") as dram,
            tc.tile_pool(name="adj", bufs=4) as adjp,
            tc.tile_pool(name="small", bufs=4) as smallp,
        ):
            # ---- constants ----
            w2t = constp.tile([128, HID // 128, OUT], BF16)
            nc.sync.dma_start(w2t[:], w2[:].rearrange("(f p) n -> p f n", p=128))
            b1t = constp.tile([128, HID // 128], F32)
            nc.sync.dma_start(b1t[:], b1T[:])
            b2t = constp.tile([128, OUT // 128], F32)
            nc.sync.dma_start(b2t[:], b2T[:])

            # AllGather 1 split by (n-half, row-half): 4 collectives of
            # [1024, 512].  Row-halving lets B start on the k-blocks of the
            # first row-half while the second still gathers — and phase A's
            # second row-half fills the PE during the first gather.
            ag1_in = [
                [dram.tile([SH // 2, 512], F8, name=f"ag1i{h}{rh}") for rh in range(2)]
                for h in range(2)
            ]
            ag1_out = [
                [
                    dram.tile([N // 2, 512], F8, addr_space="Shared",
                              name=f"ag1o{h}{rh}")
                    for rh in range(2)
                ]
                for h in range(2)
            ]
            # AllGather 2 split in eighths ([256, OUT] each) so phase D's
            # final k-blocks aren't gated on one big late collective.
            ag2_in = [dram.tile([SH // 8, OUT], F8, name=f"ag2i{e}") for e in range(8)]
            ag2_out = [
                dram.tile([N // 8, OUT], F8, addr_space="Shared", name=f"ag2o{e}")
                for e in range(8)
            ]

            # ---- phase A: s1_c = x_c @ W1 (row-halves; AG per (h, rh)) ----
            with tc.tile_pool(name="phA", bufs=1) as pA:
                xt = []
                w1t = []
                for f in range(4):
                    t = pA.tile([128, SH], BF16, name=f"xt{f}")
                    nc.sync.dma_start(t[:], xT[f * 128 : (f + 1) * 128, :])
                    xt.append(t)
                    t = pA.tile([128, HID], BF16, name=f"w1t{f}")
                    nc.sync.dma_start(t[:], w1[f * 128 : (f + 1) * 128, :])
                    w1t.append(t)
                for rh in range(2):
                    for mth in range(SH // 256):
                        mt = rh * (SH // 256) + mth
                        psa = [
                            psum.tile([128, 512], F32, tag="ps", name=f"psA{h}{mt}")
                            for h in range(2)
                        ]
                        # both n-halves per stationary x slice (shared LDW)
                        for f in range(4):
                            for h in range(2):
                                nc.tensor.matmul(
                                    psa[h][:],
                                    xt[f][:, mt * 128 : (mt + 1) * 128],
                                    w1t[f][:, h * 512 : (h + 1) * 512],
                                    start=(f == 0),
                                    stop=(f == 3),
                                )
                        for h in range(2):
                            s1o = smallp.tile([128, 512], F8, tag="s1o", bufs=2)
                            nc.scalar.activation(
                                s1o[:],
                                psa[h][:],
                                mybir.ActivationFunctionType.Copy,
                                scale=S_S1,
                            )
                            nc.scalar.dma_start(
                                ag1_in[h][rh][mth * 128 : (mth + 1) * 128, :], s1o[:]
                            )
                    # n-half 0 gathers immediately (it gates phase B's
                    # start); n-half 1 is gathered later (delayed dep)
                    allgather(ag1_in[0][rh], ag1_out[0][rh])

            # ---- phases B + C (C quarters interleaved so AG2 fires early) --
            with (
                tc.tile_pool(name="s1res", bufs=32) as s1p,
                tc.tile_pool(name="ht", bufs=32) as htp,
            ):
                ht_tiles = {}

                def phase_c_quarter(qq):
                    # s2 rows qq*512 .. +511 (needs ht tiles mb=qq, all f);
                    # gather per eighth (every 2 row-tiles of 128)
                    for mth in range(4):
                        mt = qq * 4 + mth
                        mb, off = mt // 4, (mt % 4) * 128
                        psc = psum.tile([128, OUT], F32, tag="ps", name=f"psC{mt}")
                        for f in range(8):
                            nc.tensor.matmul(
                                psc[:],
                                ht_tiles[(f, mb)][:, off : off + 128],
                                w2t[:, f, :],
                                start=(f == 0),
                                stop=(f == 7),
                            )
                        s2o = smallp.tile([128, OUT], F8, tag="s2o", bufs=2)
                        nc.scalar.activation(
                            s2o[:],
                            psc[:],
                            mybir.ActivationFunctionType.Copy,
                            scale=S_S2,
                        )
                        e = qq * 2 + mth // 2
                        nc.scalar.dma_start(
                            ag2_in[e][(mth % 2) * 128 : (mth % 2) * 128 + 128, :],
                            s2o[:],
                        )
                        if mth % 2 == 1:
                            allgather(ag2_in[e], ag2_out[e])

                # B k-block (512 rows) stream order: row-half rh, then tile t
                # within the (h, rh) gather buffer.  Buffer rows are
                # g*1024 + kb*512 + r (rank g), so tile t = g*2 + kb maps to
                # global k4 = g*4 + rh*2 + kb.
                def k4_of(rh, t):
                    return (t // 2) * 4 + rh * 2 + (t % 2)

                for nh in range(2):
                    s1_srcs = [
                        ag1_out[nh][rh][:].rearrange(
                            "(t kk p) n -> p t kk n", kk=4, p=128
                        )
                        for rh in range(2)
                    ]
                    s1t = {}
                    for mbp in range(2):
                        ps = [
                            psum.tile(
                                [128, 512], F32, tag="ps", name=f"psB{nh}{mbp}{i}"
                            )
                            for i in range(8)
                        ]  # index nt*2 + mbx
                        for rh in range(2):
                            for tt in range(16):
                                k4g = k4_of(rh, tt)
                                if mbp == 0:
                                    t = s1p.tile(
                                        [128, 4, 512], F8, tag="s1t",
                                        name=f"s1t{nh}{rh}{tt}",
                                    )
                                    nc.sync.dma_start(t[:], s1_srcs[rh][:, tt])
                                    s1t[(rh, tt)] = t
                                atp = adjp.tile(
                                    [128, 4, 1024], F8, tag="adjt", bufs=4,
                                    name=f"at{nh}{mbp}{rh}{tt}",
                                )
                                nc.sync.dma_start(
                                    atp[:],
                                    adjT[
                                        k4g * 512 : (k4g + 1) * 512,
                                        mbp * 1024 : (mbp + 1) * 1024,
                                    ].rearrange("(kk p) m -> p kk m", p=128),
                                )
                                for kkp in range(2):
                                    for nt in range(4):
                                        lhs = s1t[(rh, tt)][
                                            :, 2 * kkp : 2 * kkp + 2,
                                            nt * 128 : (nt + 1) * 128,
                                        ]
                                        for mbx in range(2):
                                            nc.tensor.matmul(
                                                ps[nt * 2 + mbx][:],
                                                lhs,
                                                atp[
                                                    :, 2 * kkp : 2 * kkp + 2,
                                                    mbx * 512 : (mbx + 1) * 512,
                                                ],
                                                start=(
                                                    rh == 0 and tt == 0 and kkp == 0
                                                ),
                                                stop=(
                                                    rh == 1 and tt == 15 and kkp == 1
                                                ),
                                                perf_mode=DR,
                                            )
                        last_act = None
                        for nt in range(4):
                            j = nh * 4 + nt
                            for mbx in range(2):
                                mb = mbp * 2 + mbx
                                htt = htp.tile([128, 512], BF16, tag="htt")
                                last_act = nc.scalar.activation(
                                    htt[:],
                                    ps[nt * 2 + mbx][:],
                                    mybir.ActivationFunctionType.Relu,
                                    bias=b1t[:, j : j + 1],
                                    scale=B_DESCALE,
                                )
                                ht_tiles[(j, mb)] = htt
                        if nh == 0 and mbp == 0:
                            # fire the second n-half s1 gathers now; dep
                            # delays their traffic past B's startup loads
                            for rh in range(2):
                                cc = allgather(ag1_in[1][rh], ag1_out[1][rh])
                                tile_rust_add_dep(
                                    cc.ins,
                                    last_act.ins,
                                    sync=True,
                                    reason="delay s1 half-1 gather past B start",
                                )
                        if nh == 1:
                            # ht tiles for mb 2*mbp..2*mbp+1 now complete for
                            # all f -> emit the matching C quarters + gathers.
                            phase_c_quarter(2 * mbp)
                            phase_c_quarter(2 * mbp + 1)

            # ---- phase D: out2T = (adj_c @ s2)^T + b2 ----
            # All 8 psum banks accumulate concurrently; k-blocks (256 rows =
            # one DR pair) consumed in gather-arrival order (eighth-major);
            # s2 tiles loaded JIT after each adjT chunk so the SP queue stays
            # load-ordered.
            with (
                tc.tile_pool(name="s2res", bufs=32) as s2p,
                tc.tile_pool(name="adjD", bufs=8) as adjDp,
                tc.tile_pool(name="outp", bufs=8) as outp,
            ):
                # ag2_out[e] rows = g*256 + kk*128 + p  (rank g, eighth e)
                s2_srcs = [
                    ag2_out[e][:].rearrange("(g kk p) n -> p g kk n", g=8, p=128)
                    for e in range(8)
                ]
                dps = [
                    psum.tile([128, 512], F32, tag="ps", name=f"psD{i}")
                    for i in range(8)
                ]
                # k2 = g*8 + e  ->  iterate eighth-major
                k2_order = [g * 8 + e for e in range(8) for g in range(8)]
                for ki, k2 in enumerate(k2_order):
                    g, e = k2 // 8, k2 % 8
                    at = adjDp.tile([128, 2, SH], F8, tag="adjD", name=f"atD{k2}")
                    nc.sync.dma_start(
                        at[:],
                        adjT[k2 * 256 : (k2 + 1) * 256, :].rearrange(
                            "(kk p) m -> p kk m", p=128
                        ),
                    )
                    st = s2p.tile([128, 2, OUT], F8, tag="s2t", name=f"s2t{k2}")
                    nc.sync.dma_start(st[:], s2_srcs[e][:, g])
                    for n2t in range(2):
                        lhs = st[:, :, n2t * 128 : (n2t + 1) * 128]
                        for mb in range(4):
                            nc.tensor.matmul(
                                dps[n2t * 4 + mb][:],
                                lhs,
                                at[:, :, mb * 512 : (mb + 1) * 512],
                                start=(ki == 0),
                                stop=(ki == 63),
                                perf_mode=DR,
                            )
                # drain the 8 psum banks on alternating engines (ACT + DVE;
                # GpSimd cannot access PSUM) so the final copies pipeline
                mult, add = mybir.AluOpType.mult, mybir.AluOpType.add
                for n2t in range(2):
                    for mb in range(4):
                        ot = outp.tile([128, 512], F32, tag="ot")
                        if (n2t * 4 + mb) % 2 == 0:
                            nc.scalar.activation(
                                ot[:],
                                dps[n2t * 4 + mb][:],
                                mybir.ActivationFunctionType.Identity,
                                bias=b2t[:, n2t : n2t + 1],
                                scale=D_DESCALE,
                            )
                        else:
                            nc.vector.tensor_scalar(
                                ot[:],
                                dps[n2t * 4 + mb][:],
                                D_DESCALE,
                                b2t[:, n2t : n2t + 1],
                                mult,
                                add,
                            )
                        nc.scalar.dma_start(
                            out2T[
                                n2t * 128 : (n2t + 1) * 128, mb * 512 : (mb + 1) * 512
                            ],
                            ot[:],
                        )

    _elide_redundant_ldweights(nc)
    _split_excess_waits(nc)
    return nc


def _prep_inputs(x, adj, W1, b1, W2, b2):
    bf = ml_dtypes.bfloat16
    w1b = W1.astype(bf)
    w2b = W2.astype(bf)
    b1T = np.ascontiguousarray(b1.reshape(HID // 128, 128).T).astype(np.float32)
    b2T = np.ascontiguousarray(b2.reshape(OUT // 128, 128).T).astype(np.float32)
    in_maps = []
    for c in range(NCORES):
        rows = slice(c * SH, (c + 1) * SH)
        adj8 = np.clip(adj[rows, :].T * np.float32(S_ADJ), -240.0, 240.0).astype(NPF8)
        in_maps.append(
            {
                "adjT": adj8,
                "xT": x[rows, :].T.astype(bf),
                "w1": w1b,
                "w2": w2b,
                "b1T": b1T,
                "b2T": b2T,
            }
        )
    return in_maps


def _run(inputs, trace=False):
    global _built
    if _built is None:
        _built = build()
    in_maps = _prep_inputs(**inputs)
    r = run_bass_kernel_spmd(_built, in_maps, list(range(NCORES)), trace=trace)
    out = np.empty([N, OUT], np.float32)
    for c in range(NCORES):
        out[c * SH : (c + 1) * SH, :] = r.results[c]["out2T"].T
    return out, r


def kernel(x, adj, W1, b1, W2, b2):
    out, _ = _run(dict(x=x, adj=adj, W1=W1, b1=b1, W2=W2, b2=b2))
    return out
